# revision 1
# baseline (speedup 1.0000x reference)
"""AttnTopKPool Trainium2 kernel.

reference:
    w_mean = mean(w, axis=1)          # [B, S, S] -> [B, S]
    idx    = top_k(w_mean, 16)        # [B, 16]
    out    = x[b, :, idx[b]]          # [B, F, 16]

Strategy (8 NeuronCores, batch-parallel, 4 batches each):
  - host: transpose x to x_t[b, s, f] so the device gather is a contiguous
    row gather; slice w and x_t per core.
  - device per batch (w[b] is 16 MiB, streamed once; memory-bound):
      * top-k needs exact fp32 sums: the smallest top-16 gap on these
        U(0,1) column sums is ~4e-3 while fp32r/TF32 matmul error is
        ~5e-3 (HW-probed), so the reduction stays fp32 end to end.
      * with all 8 cores streaming, the per-core HBM share is ~334 GB/s
        (HW-measured from DMA-completion cadence: 12.55 us per 4 MiB
        load, 6.3 us per 2 MiB, both = 334 GB/s; 8 x 334 = 2.7 TB/s =
        the chip HBM wall). DMA size and queue count don't change it -
        dual-HWDGE-queue and 2 MiB-load variants measured the same or
        worse. Stream floor: 64 MiB / 334 GB/s = 192 us. Four 4 MiB
        [128, 8192] loads per batch (4 w-rows/partition) on the sync
        queue; batch windows are ~50 us so every engine has slack.
      * DVE pair-adds L0+L1 -> pa0, L2+L3 -> pa1 (8.7 us each, 17.3
        us/batch; + 11.5 us topk ~= 29 us of the ~50 us batch window).
        TensorE consumes pa0/pa1: 32 fp32 [128,512] matmuls into one
        [1, 2048] PSUM tile (bank = chunk mod 4); ~29 us even with the
        PE clock-gate cold. top-k of sum == top-k of mean.
      * batch 3 (the tail batch): L0,L1 pair-added; L2 and four 1 MiB
        [128, 2048] small loads fed to PE directly, so the post-stream
        chain is 4 small matmuls - no 8.7 us add gates the tail.
      * top-16 via DVE max8 / max_index straight out of PSUM (the SBUF
        copy for match_replace runs in parallel on scalar); first 8
        gathers issue between the two passes.
      * gather: per index, reg_load into a register and issue a
        dynamic-offset DMA copying that 4 KiB row of x_t[b] straight
        DRAM->DRAM into the output (no SBUF bounce), on scalar+gpsimd
        (never the streaming sync queue; +sync for the last batch once
        the stream is done).
  - out per core: [64, 1024] = (b_loc*16 + k, f); host reassembles to
    [B, F, K].
"""

import numpy as np

B, F, S, K = 32, 1024, 2048, 16
N_CORES = 8
B_LOC = B // N_CORES  # 4
P = 128
ROWS_PER_PART = 4          # w rows per SBUF partition in one big load
LOAD_FREE = ROWS_PER_PART * S   # 8192 floats = 32 KiB per partition
MM_N = 512                 # one PSUM bank of fp32
NQ = S // MM_N             # 4 psum column slices
NEG = -3.0e38              # below any column sum

_cached_nc = None

# test-only knobs (harness leaves these at defaults)
TRACE = False
_last_results = None


def _build_nc():
    from concourse import bacc, bass, mybir, tile

    f32 = mybir.dt.float32
    u32 = mybir.dt.uint32

    nc = bacc.Bacc("TRN2", target_bir_lowering=False, debug=False)

    w_d = nc.dram_tensor("w", [B_LOC, S, S], f32, kind="ExternalInput")
    xt_d = nc.dram_tensor("xt", [B_LOC, S, F], f32, kind="ExternalInput")
    out_d = nc.dram_tensor("out", [B_LOC * K, F], f32, kind="ExternalOutput")

    w_rows = w_d[:].rearrange("b r s -> (b r) s")
    # big view: [16, 128, 8192]; partition p of slot t holds rows (512t + 4p ..+3)
    w_big = w_rows.rearrange("(t p fr) s -> t p (fr s)", p=P, fr=ROWS_PER_PART)
    # small view: [64, 128, 2048]; partition p of slot m holds row (128m + p)
    w_small = w_rows.rearrange("(m p) s -> m p s", p=P)
    LAST = B_LOC - 1
    N_CHUNK = LOAD_FREE // MM_N  # 16 matmul chunks per pair tile

    with tile.TileContext(nc) as tc:
        with (
            tc.tile_pool(name="wpool", bufs=3) as wpool,
            tc.tile_pool(name="papool", bufs=2) as papool,
            tc.tile_pool(name="stpool", bufs=2) as stpool,
            tc.tile_pool(name="smpool", bufs=1) as smpool,
            tc.tile_pool(name="pspool", bufs=2, space="PSUM") as pspool,
            tc.tile_pool(name="tk", bufs=1) as tk,
        ):
            ones = tk.tile([P, 1], f32)
            nc.vector.memset(ones[:], 1.0)

            for b in range(B_LOC):
                ps = pspool.tile([1, S], f32, name=f"ps{b}", tag="ps")
                n_pairs = 1 if b == LAST else 2

                for i in range(n_pairs):
                    wa = wpool.tile([P, LOAD_FREE], f32, name=f"wa{b}_{i}", tag="wt")
                    nc.sync.dma_start(wa[:], w_big[b * 4 + 2 * i])
                    wb = wpool.tile([P, LOAD_FREE], f32, name=f"wb{b}_{i}", tag="wt")
                    nc.sync.dma_start(wb[:], w_big[b * 4 + 2 * i + 1])
                    pa = papool.tile([P, LOAD_FREE], f32, name=f"pa{b}_{i}", tag="pa")
                    # the last batch's add is halved so its PE backlog (it
                    # gates the kernel tail) starts ~4 us earlier
                    halves = 2 if b == LAST else 1
                    hn = LOAD_FREE // halves
                    for h in range(halves):
                        nc.vector.tensor_add(
                            pa[:, h * hn : (h + 1) * hn],
                            wa[:, h * hn : (h + 1) * hn],
                            wb[:, h * hn : (h + 1) * hn],
                        )
                        for c in range(h * hn // MM_N, (h + 1) * hn // MM_N):
                            q = c % NQ
                            nc.tensor.matmul(
                                ps[:, q * MM_N : (q + 1) * MM_N],
                                ones[:],
                                pa[:, c * MM_N : (c + 1) * MM_N],
                                start=(i == 0 and c < NQ),
                                stop=(
                                    b != LAST and i == 1 and c >= N_CHUNK - NQ
                                ),
                            )
                if b == LAST:
                    # L2 direct to PE, then four 1 MiB smalls pair-added:
                    # the PE backlog that gates the tail drops from 48 to
                    # 40 matmuls, and the post-stream chain is one 2.2 us
                    # small add + 4 small matmuls
                    wc = wpool.tile([P, LOAD_FREE], f32, name=f"wc{b}", tag="wt")
                    nc.sync.dma_start(wc[:], w_big[b * 4 + 2])
                    for c in range(N_CHUNK):
                        q = c % NQ
                        nc.tensor.matmul(
                            ps[:, q * MM_N : (q + 1) * MM_N],
                            ones[:],
                            wc[:, c * MM_N : (c + 1) * MM_N],
                            start=False,
                            stop=False,
                        )
                    for j in range(2):
                        sa = stpool.tile([P, S], f32, name=f"sa{j}", tag="st", bufs=3)
                        nc.sync.dma_start(sa[:], w_small[b * 16 + 12 + 2 * j])
                        sb = stpool.tile([P, S], f32, name=f"sb{j}", tag="st", bufs=3)
                        nc.sync.dma_start(sb[:], w_small[b * 16 + 13 + 2 * j])
                        pas = stpool.tile([P, S], f32, name=f"pas{j}", tag="pas", bufs=1)
                        nc.vector.tensor_add(pas[:], sa[:], sb[:])
                        for q in range(NQ):
                            nc.tensor.matmul(
                                ps[:, q * MM_N : (q + 1) * MM_N],
                                ones[:],
                                pas[:, q * MM_N : (q + 1) * MM_N],
                                start=False,
                                stop=(j == 1),
                            )

                # --- top-16, first 8 gathers overlap the second pass ---
                # every op reads PSUM directly; match_replace streams
                # PSUM -> SBUF applying the replacement, so no separate
                # copy sits anywhere in the chain
                sums = smpool.tile([1, S], f32, name=f"sums{b}", tag="sums")
                gidx = tk.tile([1, K], u32, name=f"gidx{b}")
                m8a = tk.tile([1, 8], f32, name=f"m8a{b}")
                nc.vector.max(m8a[:], ps[:])
                nc.vector.max_index(gidx[:, 0:8], m8a[:], ps[:])

                def gather(k, etype, eng):
                    regs = nc.alloc_registers(name=f"ri{b}_{k}", engines=(etype,))
                    reg = list(regs)[0]
                    eng.reg_load(reg, gidx[0:1, k : k + 1])
                    val = eng.snap(reg, donate=True, min_val=0, max_val=S - 1)
                    eng.dma_start(
                        out_d[b * K + k : b * K + k + 1, :],
                        xt_d[b][bass.ds(val, 1), :],
                    )

                engs = [
                    (mybir.EngineType.Activation, nc.scalar),
                    (mybir.EngineType.Pool, nc.gpsimd),
                ]
                if b == LAST:
                    engs.append((mybir.EngineType.SP, nc.sync))
                for k in range(8):
                    gather(k, *engs[k % len(engs)])

                m8b = tk.tile([1, 8], f32, name=f"m8b{b}")
                nc.vector.match_replace(sums[:], m8a[:], ps[:], NEG)
                nc.vector.max(m8b[:], sums[:])
                nc.vector.max_index(gidx[:, 8:16], m8b[:], sums[:])
                for k in range(8, K):
                    gather(k, *engs[k % len(engs)])

    nc.compile()
    return nc


def _get_nc():
    global _cached_nc
    if _cached_nc is None:
        _cached_nc = _build_nc()
    return _cached_nc


def kernel(x: np.ndarray, w: np.ndarray) -> np.ndarray:
    from concourse import bass_utils

    x = np.asarray(x, dtype=np.float32)
    w = np.asarray(w, dtype=np.float32)
    x_t = np.ascontiguousarray(x.transpose(0, 2, 1))  # [B, S, F]

    nc = _get_nc()
    in_maps = [
        {
            "w": np.ascontiguousarray(w[c * B_LOC : (c + 1) * B_LOC]),
            "xt": x_t[c * B_LOC : (c + 1) * B_LOC],
        }
        for c in range(N_CORES)
    ]
    res = bass_utils.run_bass_kernel_spmd(
        nc, in_maps, list(range(N_CORES)), trace=TRACE
    )
    global _last_results
    _last_results = res
    out = np.concatenate([res.results[c]["out"] for c in range(N_CORES)], axis=0)
    # [B*K, F] -> [B, K, F] -> [B, F, K]
    return np.ascontiguousarray(out.reshape(B, K, F).transpose(0, 2, 1))

